# revision 38
# baseline (speedup 1.0000x reference)
"""Trainium2 (8 NeuronCores) kernel for single-head causal attention.

Problem: x [8, 2048, 1024] f32; Wq/Wk/Wv [1024, 128] f32.
    q = x @ Wq ; k = x @ Wk ; v = x @ Wv          (per batch row)
    out = softmax(causal(q @ k^T / sqrt(128))) @ v  -> [8, 2048, 128] f32

Sharding: pure data-parallel — one batch row per NeuronCore, weights
replicated. No collectives.

Per-core algorithm (bf16 matmul inputs, f32 PSUM accumulation):
  Host supplies xT = x[b].T  [D, T] in bf16 (layout prep only).
  A) qT/kT [H=128 part, T] with W-chunks stationary over 8 D-chunks;
     v [T-block part, H] directly with xT chunks stationary. An all-ones
     column is appended to v so the softmax denominator falls out of the
     output matmul for free.
  B) Scores computed TRANSPOSED: sT[k,q] = kT_j-block-stationary @ qT,
     causally block-skipped (only q >= 512-aligned start of k-block j).
     exp(scale*s) runs on ScalarE straight out of PSUM into bf16 wT tiles
     (no max-subtraction: |scale*s| <= ~7 here, safely in f32/bf16 range).
     Diagonal 128x128 blocks additionally get a multiplicative 0/1
     strictly-causal mask applied post-exp into separate diag tiles.
  C) out[q,h] accumulates over k-blocks j<=i with wT blocks stationary and
     v_aug moving (N=129). Column 128 of PSUM is l = sum_k w; normalize
     with one reciprocal + ScalarE copy-with-per-partition-scale, DMA out.

Engine discipline: every matmul may carry at most ONE semaphore wait, so
all PSUM readers are pinned to ScalarE (copies, exp, epilogue scale);
the only DVE-written tiles matmuls consume are the dedicated diag tiles.
"""

from contextlib import ExitStack

import ml_dtypes
import numpy as np

B, T, D, H = 8, 2048, 1024, 128
P = 128
DC = D // P  # 8 contraction chunks
TB = T // P  # 16 token blocks
QG = T // 512  # 4 512-wide token groups
SCALE = 1.0 / float(np.sqrt(H))

_CACHE = {}
LAST_RESULT = None


def _build():
    import concourse.bacc as bacc
    import concourse.mybir as mybir
    import concourse.tile as tile

    f32 = mybir.dt.float32
    bf16 = mybir.dt.bfloat16
    EXP = mybir.ActivationFunctionType.Exp
    MULT = mybir.AluOpType.mult

    nc = bacc.Bacc()
    xT_h = nc.declare_dram_parameter("xT", [D, T], bf16, isOutput=False)
    wq_h = nc.declare_dram_parameter("Wq", [D, H], bf16, isOutput=False)
    wk_h = nc.declare_dram_parameter("Wk", [D, H], bf16, isOutput=False)
    wv_h = nc.declare_dram_parameter("Wv", [D, H], bf16, isOutput=False)
    mask_h = nc.declare_dram_parameter("mask", [P, P], bf16, isOutput=False)
    out_h = nc.declare_dram_parameter("out", [T, H], f32, isOutput=True)

    with tile.TileContext(nc) as tc:
        with ExitStack() as ctx:
            singles = ctx.enter_context(tc.tile_pool(name="singles", bufs=1))

            xT_sb = singles.tile([P, DC, T], bf16)
            wq_sb = singles.tile([P, DC, H], bf16)
            wk_sb = singles.tile([P, DC, H], bf16)
            wv_sb = singles.tile([P, DC, H], bf16)
            mask_sb = singles.tile([P, P], bf16)
            mask2_sb = singles.tile([P, P], bf16)
            qT_sb = singles.tile([P, T], bf16)
            kT_sb = singles.tile([P, T], bf16)
            v_sb = singles.tile([P, TB, 132], bf16)  # [...,128] = ones col
            wT_sb = singles.tile([P, TB, T], bf16)
            dw_sb = singles.tile([P, TB, P], bf16)  # masked diagonal blocks
            # per-iteration epilogue slices (no pool recycling -> no WAR waits)
            rec_all = singles.tile([P, TB], f32)
            ot_all = singles.tile([P, TB, H], f32)

            # xT chunks as 8 FIFO DMAs on the SP ring, issued FIRST: same-queue
            # DMAs stream sequentially, so chunk 0 completes ~2 us in and the
            # d-outer projection loop tracks the input as it lands. (Parallel
            # queues round-robin at packet granularity — every chunk would
            # finish late together.) Weights ride the ACT ring concurrently.
            xT_ap = xT_h[:]
            for c in range(DC):
                nc.sync.dma_start(
                    out=xT_sb[:, c, :], in_=xT_ap[c * P : (c + 1) * P, :]
                )
            for w_h, w_sb in ((wq_h, wq_sb), (wk_h, wk_sb), (wv_h, wv_sb)):
                nc.scalar.dma_start(
                    out=w_sb, in_=w_h[:].rearrange("(c p) h -> p c h", p=P)
                )
            nc.scalar.dma_start(out=mask_sb, in_=mask_h[:])
            # ACT pre-touch: moves the mask's DMA wait onto a junction copy so
            # the per-j diag multiply's two deps (exp + mask) merge into one
            # ACT wait — hardware instructions carry at most one sem wait.
            nc.scalar.copy(mask2_sb, mask_sb)

            # PE warm-up fodder: HAM starts the PE throttled at 1.2 GHz and
            # needs ~3.4 us of sustained work to unthrottle; these dummies run
            # in the launch/DMA dead window. They write qps[0] BEFORE its real
            # accumulation group begins (start=True clears the bank).
            warm_sb = singles.tile([P, 512], bf16)
            nc.vector.memset(warm_sb, 0.0)

            # --- Phase A1: q/k projections, d-chunk OUTER so each xT chunk is
            # consumed as its DMA lands (PE overlaps the input load). 8 PSUM
            # banks live at once; pool scoped so phase B/C reuse the space.
            with tc.tile_pool(name="psQK", bufs=1, space="PSUM") as psQK:
                qps = [
                    psQK.tile([P, 512], f32, tag=f"q{g}", name=f"q{g}")
                    for g in range(QG)
                ]
                kps = [
                    psQK.tile([P, 512], f32, tag=f"k{g}", name=f"k{g}")
                    for g in range(QG)
                ]
                for _ in range(8):
                    nc.tensor.matmul(
                        qps[0], warm_sb[:, 0:128], warm_sb, start=True, stop=True
                    )

                def qk_mm(is_q, g, c):
                    w_sb = wq_sb if is_q else wk_sb
                    tiles = qps if is_q else kps
                    nc.tensor.matmul(
                        tiles[g],
                        w_sb[:, c, :],
                        xT_sb[:, c, g * 512 : (g + 1) * 512],
                        start=(c == 0),
                        stop=(c == DC - 1),
                    )

                for c in range(DC - 1):
                    for is_q in (True, False):
                        for g in range(QG):
                            qk_mm(is_q, g, c)
                # Last chunk: emit each stop-matmul immediately followed by its
                # PSUM->SBUF copy, ordered so the first exps' inputs (qT then
                # kT block group 0, which covers k-blocks 0..3) land first —
                # the copies pipeline on DVE behind the remaining matmuls.
                for is_q, g in (
                    (True, 0),
                    (True, 1),
                    (False, 0),
                    (True, 2),
                    (True, 3),
                    (False, 1),
                    (False, 2),
                    (False, 3),
                ):
                    qk_mm(is_q, g, DC - 1)
                    dst = qT_sb if is_q else kT_sb
                    src = qps[g] if is_q else kps[g]
                    nc.vector.tensor_copy(dst[:, g * 512 : (g + 1) * 512], src)

            with ExitStack() as ctx2:
                psS = ctx2.enter_context(
                    tc.tile_pool(name="psS", bufs=2, space="PSUM")
                )
                psV = ctx2.enter_context(
                    tc.tile_pool(name="psV", bufs=2, space="PSUM")
                )
                psO = ctx2.enter_context(
                    tc.tile_pool(name="psO", bufs=2, space="PSUM")
                )

                # ones column of v_aug, once (region disjoint from v copies)
                nc.vector.memset(v_sb[:, :, 128:129], 1.0)

                # --- Phases B+A2+C, software-pipelined by one j: per k-block
                # j emit its score matmuls + exp + v projection, then output
                # group C_{j-1}, whose inputs (exps/dw/v for blocks <= j-1)
                # are all complete by then — so C's matmuls carry no waits and
                # the PE stream stays dense while ScalarE exps run alongside.
                out_ap = out_h[:]

                def emit_c_group(i):
                    po = psO.tile([P, 132], f32, tag="psO", name=f"po{i}")
                    for jj in range(i):
                        nc.tensor.matmul(
                            po[:, 0:129],
                            wT_sb[:, jj, i * P : (i + 1) * P],
                            v_sb[:, jj, 0:129],
                            start=(jj == 0),
                            stop=False,
                        )
                    nc.tensor.matmul(
                        po[:, 0:129],
                        dw_sb[:, i, :],
                        v_sb[:, i, 0:129],
                        start=(i == 0),
                        stop=True,
                    )
                    nc.vector.reciprocal(rec_all[:, i : i + 1], po[:, 128:129])
                    nc.scalar.mul(ot_all[:, i, :], po[:, 0:H], rec_all[:, i : i + 1])
                    # Issue from the ACT sequencer: the data dep on ot (ACT
                    # mul) is program-order, leaving only the ring-FIFO wait.
                    nc.scalar.dma_start(
                        out=out_ap[i * P : (i + 1) * P, :], in_=ot_all[:, i, :]
                    )

                for j in range(TB):
                    # scores (transposed), exact-causal: q in [j*128, T)
                    L = T - j * P
                    kT_j = kT_sb[:, j * P : (j + 1) * P]
                    if j == 0:
                        # small first piece so exp_0 starts after only 3 copies
                        pieces = [(0, 512), (512, 1024), (1536, 512)]
                    else:
                        pieces = [
                            (b, min(1024, L - b)) for b in range(0, L, 1024)
                        ]
                    for base, cw in pieces:
                        ps = psS.tile([P, 1024], f32, tag="psS")
                        for off in range(0, cw, 512):
                            w = min(512, cw - off)
                            nc.tensor.matmul(
                                ps[:, off : off + w],
                                kT_j,
                                qT_sb[:, j * P + base + off : j * P + base + off + w],
                                start=True,
                                stop=True,
                            )
                        nc.scalar.activation(
                            wT_sb[:, j, j * P + base : j * P + base + cw],
                            ps[:, :cw],
                            EXP,
                            scale=SCALE,
                        )
                    nc.vector.tensor_tensor(
                        dw_sb[:, j, :], wT_sb[:, j, j * P : (j + 1) * P], mask2_sb, MULT
                    )

                    # v projection for block j
                    pv = psV.tile([P, H], f32, tag="psV")
                    for c in range(DC):
                        nc.tensor.matmul(
                            pv,
                            xT_sb[:, c, j * P : (j + 1) * P],
                            wv_sb[:, c, :],
                            start=(c == 0),
                            stop=(c == DC - 1),
                        )
                    nc.vector.tensor_copy(v_sb[:, j, 0:H], pv)

                    if j > 0:
                        emit_c_group(j - 1)
                emit_c_group(TB - 1)

    _strip_self_waits(nc)
    nc.finalize()  # Bacc.compile(): wait legalization + register allocation
    return nc


def _strip_self_waits(nc):
    """Drop same-engine semaphore waits on in-order engines (PE/ACT/DVE
    execute and complete strictly in order, so a self-wait is redundant).
    Tile emits them conservatively; walrus allows only one sem wait per
    compute instruction, and these push some matmuls/tensor-ops over."""
    prefixes = {"PE": "PE_", "Activation": "Activation_", "DVE": "DVE_"}
    for bb in nc.m.functions[0].blocks:
        for inst in bb.instructions:
            si = inst.sync_info
            if not si or not si.on_wait:
                continue
            pref = prefixes.get(str(inst.engine).split(".")[-1])
            if pref is None:
                continue
            keep = [w for w in si.on_wait if not (w.ant_name or "").startswith(pref)]
            if len(keep) != len(si.on_wait):
                si.on_wait = keep
                inst.sync_info = si


def kernel(**inputs):
    global LAST_RESULT
    x = np.asarray(inputs["x"], dtype=np.float32)
    bf = ml_dtypes.bfloat16
    w_bf = {
        k: np.asarray(inputs[k], dtype=np.float32).astype(bf)
        for k in ("Wq", "Wk", "Wv")
    }
    # dw[p=k_local, f=q_local] keeps entries with k <= q
    mask01 = (
        (np.arange(P)[:, None] <= np.arange(P)[None, :]).astype(np.float32).astype(bf)
    )

    if "nc" not in _CACHE:
        _CACHE["nc"] = _build()
    nc = _CACHE["nc"]

    from concourse.bass_utils import run_bass_kernel_spmd

    in_maps = [
        {
            "xT": np.ascontiguousarray(x[b].T).astype(bf),
            "Wq": w_bf["Wq"],
            "Wk": w_bf["Wk"],
            "Wv": w_bf["Wv"],
            "mask": mask01,
        }
        for b in range(B)
    ]
    res = run_bass_kernel_spmd(nc, in_maps, core_ids=list(range(B)))
    LAST_RESULT = res
    return np.stack([res.results[b]["out"] for b in range(B)]).astype(np.float32)


# revision 39
# speedup vs baseline: 1.0143x; 1.0143x over previous
"""Trainium2 (8 NeuronCores) kernel for single-head causal attention.

Problem: x [8, 2048, 1024] f32; Wq/Wk/Wv [1024, 128] f32.
    q = x @ Wq ; k = x @ Wk ; v = x @ Wv          (per batch row)
    out = softmax(causal(q @ k^T / sqrt(128))) @ v  -> [8, 2048, 128] f32

Sharding: pure data-parallel — one batch row per NeuronCore, weights
replicated. No collectives.

Per-core algorithm (bf16 matmul inputs, f32 PSUM accumulation):
  Host supplies xT = x[b].T  [D, T] in bf16 (layout prep only).
  A) qT/kT [H=128 part, T] with W-chunks stationary over 8 D-chunks;
     v [T-block part, H] directly with xT chunks stationary. An all-ones
     column is appended to v so the softmax denominator falls out of the
     output matmul for free.
  B) Scores computed TRANSPOSED: sT[k,q] = kT_j-block-stationary @ qT,
     causally block-skipped (only q >= 512-aligned start of k-block j).
     exp(scale*s) runs on ScalarE straight out of PSUM into bf16 wT tiles
     (no max-subtraction: |scale*s| <= ~7 here, safely in f32/bf16 range).
     Diagonal 128x128 blocks additionally get a multiplicative 0/1
     strictly-causal mask applied post-exp into separate diag tiles.
  C) out[q,h] accumulates over k-blocks j<=i with wT blocks stationary and
     v_aug moving (N=129). Column 128 of PSUM is l = sum_k w; normalize
     with one reciprocal + ScalarE copy-with-per-partition-scale, DMA out.

Engine discipline: every matmul may carry at most ONE semaphore wait, so
all PSUM readers are pinned to ScalarE (copies, exp, epilogue scale);
the only DVE-written tiles matmuls consume are the dedicated diag tiles.
"""

from contextlib import ExitStack

import ml_dtypes
import numpy as np

B, T, D, H = 8, 2048, 1024, 128
P = 128
DC = D // P  # 8 contraction chunks
TB = T // P  # 16 token blocks
QG = T // 512  # 4 512-wide token groups
SCALE = 1.0 / float(np.sqrt(H))

_CACHE = {}
LAST_RESULT = None


def _build():
    import concourse.bacc as bacc
    import concourse.mybir as mybir
    import concourse.tile as tile

    f32 = mybir.dt.float32
    bf16 = mybir.dt.bfloat16
    EXP = mybir.ActivationFunctionType.Exp
    MULT = mybir.AluOpType.mult

    nc = bacc.Bacc()
    xT_h = nc.declare_dram_parameter("xT", [D, T], bf16, isOutput=False)
    wq_h = nc.declare_dram_parameter("Wq", [D, H], bf16, isOutput=False)
    wk_h = nc.declare_dram_parameter("Wk", [D, H], bf16, isOutput=False)
    wv_h = nc.declare_dram_parameter("Wv", [D, H], bf16, isOutput=False)
    mask_h = nc.declare_dram_parameter("mask", [P, P], bf16, isOutput=False)
    out_h = nc.declare_dram_parameter("out", [T, H], f32, isOutput=True)

    with tile.TileContext(nc) as tc:
        with ExitStack() as ctx:
            singles = ctx.enter_context(tc.tile_pool(name="singles", bufs=1))

            xT_sb = singles.tile([P, DC, T], bf16)
            wq_sb = singles.tile([P, DC, H], bf16)
            wk_sb = singles.tile([P, DC, H], bf16)
            wv_sb = singles.tile([P, DC, H], bf16)
            mask_sb = singles.tile([P, P], bf16)
            mask2_sb = singles.tile([P, P], bf16)
            qT_sb = singles.tile([P, T], bf16)
            kT_sb = singles.tile([P, T], bf16)
            v_sb = singles.tile([P, TB, 132], bf16)  # [...,128] = ones col
            wT_sb = singles.tile([P, TB, T], bf16)
            dw_sb = singles.tile([P, TB, P], bf16)  # masked diagonal blocks
            # per-iteration epilogue slices (no pool recycling -> no WAR waits)
            rec_all = singles.tile([P, TB], f32)
            ot_all = singles.tile([P, TB, H], f32)

            # xT chunks as 8 FIFO DMAs on the SP ring, issued FIRST: same-queue
            # DMAs stream sequentially, so chunk 0 completes ~2 us in and the
            # d-outer projection loop tracks the input as it lands. (Parallel
            # queues round-robin at packet granularity — every chunk would
            # finish late together.) Weights ride the ACT ring concurrently.
            xT_ap = xT_h[:]
            for c in range(DC):
                nc.sync.dma_start(
                    out=xT_sb[:, c, :], in_=xT_ap[c * P : (c + 1) * P, :]
                )
            for w_h, w_sb in ((wq_h, wq_sb), (wk_h, wk_sb), (wv_h, wv_sb)):
                nc.scalar.dma_start(
                    out=w_sb, in_=w_h[:].rearrange("(c p) h -> p c h", p=P)
                )
            nc.scalar.dma_start(out=mask_sb, in_=mask_h[:])
            # ACT pre-touch: moves the mask's DMA wait onto a junction copy so
            # the per-j diag multiply's two deps (exp + mask) merge into one
            # ACT wait — hardware instructions carry at most one sem wait.
            nc.scalar.copy(mask2_sb, mask_sb)

            # PE warm-up fodder: HAM starts the PE throttled at 1.2 GHz and
            # needs ~3.4 us of sustained work to unthrottle; these dummies run
            # in the launch/DMA dead window. They write qps[0] BEFORE its real
            # accumulation group begins (start=True clears the bank).
            warm_sb = singles.tile([P, 512], bf16)
            nc.vector.memset(warm_sb, 0.0)

            # --- Phase A1: q/k projections, d-chunk OUTER so each xT chunk is
            # consumed as its DMA lands (PE overlaps the input load). 8 PSUM
            # banks live at once; pool scoped so phase B/C reuse the space.
            with tc.tile_pool(name="psQK", bufs=1, space="PSUM") as psQK:
                qps = [
                    psQK.tile([P, 512], f32, tag=f"q{g}", name=f"q{g}")
                    for g in range(QG)
                ]
                kps = [
                    psQK.tile([P, 512], f32, tag=f"k{g}", name=f"k{g}")
                    for g in range(QG)
                ]
                for _ in range(8):
                    nc.tensor.matmul(
                        qps[0], warm_sb[:, 0:128], warm_sb, start=True, stop=True
                    )

                def qk_mm(is_q, g, c):
                    w_sb = wq_sb if is_q else wk_sb
                    tiles = qps if is_q else kps
                    nc.tensor.matmul(
                        tiles[g],
                        w_sb[:, c, :],
                        xT_sb[:, c, g * 512 : (g + 1) * 512],
                        start=(c == 0),
                        stop=(c == DC - 1),
                    )

                for c in range(DC - 1):
                    for is_q in (True, False):
                        for g in range(QG):
                            qk_mm(is_q, g, c)
                # Last chunk: emit each stop-matmul immediately followed by its
                # PSUM->SBUF copy, ordered so the first exps' inputs (qT then
                # kT block group 0, which covers k-blocks 0..3) land first —
                # the copies pipeline on DVE behind the remaining matmuls.
                for is_q, g in (
                    (True, 0),
                    (True, 1),
                    (False, 0),
                    (True, 2),
                    (True, 3),
                    (False, 1),
                    (False, 2),
                    (False, 3),
                ):
                    qk_mm(is_q, g, DC - 1)
                    dst = qT_sb if is_q else kT_sb
                    src = qps[g] if is_q else kps[g]
                    nc.vector.tensor_copy(dst[:, g * 512 : (g + 1) * 512], src)

            with ExitStack() as ctx2:
                psS = ctx2.enter_context(
                    tc.tile_pool(name="psS", bufs=2, space="PSUM")
                )
                psV = ctx2.enter_context(
                    tc.tile_pool(name="psV", bufs=1, space="PSUM")
                )
                psO = ctx2.enter_context(
                    tc.tile_pool(name="psO", bufs=3, space="PSUM")
                )

                # ones column of v_aug, once (region disjoint from v copies)
                nc.vector.memset(v_sb[:, :, 128:129], 1.0)

                # --- Phases B+A2+C, software-pipelined by one j: per k-block
                # j emit its score matmuls + exp + v projection, then output
                # group C_{j-1}, whose inputs (exps/dw/v for blocks <= j-1)
                # are all complete by then — so C's matmuls carry no waits and
                # the PE stream stays dense while ScalarE exps run alongside.
                out_ap = out_h[:]

                def emit_c_group(i):
                    po = psO.tile([P, 132], f32, tag="psO", name=f"po{i}")
                    for jj in range(i):
                        nc.tensor.matmul(
                            po[:, 0:129],
                            wT_sb[:, jj, i * P : (i + 1) * P],
                            v_sb[:, jj, 0:129],
                            start=(jj == 0),
                            stop=False,
                        )
                    nc.tensor.matmul(
                        po[:, 0:129],
                        dw_sb[:, i, :],
                        v_sb[:, i, 0:129],
                        start=(i == 0),
                        stop=True,
                    )
                    nc.vector.reciprocal(rec_all[:, i : i + 1], po[:, 128:129])
                    nc.scalar.mul(ot_all[:, i, :], po[:, 0:H], rec_all[:, i : i + 1])
                    # Issue from the ACT sequencer: the data dep on ot (ACT
                    # mul) is program-order, leaving only the ring-FIFO wait.
                    nc.scalar.dma_start(
                        out=out_ap[i * P : (i + 1) * P, :], in_=ot_all[:, i, :]
                    )

                for j in range(TB):
                    # scores (transposed), exact-causal: q in [j*128, T)
                    L = T - j * P
                    kT_j = kT_sb[:, j * P : (j + 1) * P]
                    if j == 0:
                        # small first piece so exp_0 starts after only 3 copies
                        pieces = [(0, 512), (512, 1024), (1536, 512)]
                    else:
                        pieces = [
                            (b, min(1024, L - b)) for b in range(0, L, 1024)
                        ]
                    for base, cw in pieces:
                        ps = psS.tile([P, 1024], f32, tag="psS")
                        for off in range(0, cw, 512):
                            w = min(512, cw - off)
                            nc.tensor.matmul(
                                ps[:, off : off + w],
                                kT_j,
                                qT_sb[:, j * P + base + off : j * P + base + off + w],
                                start=True,
                                stop=True,
                            )
                        nc.scalar.activation(
                            wT_sb[:, j, j * P + base : j * P + base + cw],
                            ps[:, :cw],
                            EXP,
                            scale=SCALE,
                        )
                    nc.vector.tensor_tensor(
                        dw_sb[:, j, :], wT_sb[:, j, j * P : (j + 1) * P], mask2_sb, MULT
                    )

                    # v projection for block j
                    pv = psV.tile([P, H], f32, tag="psV")
                    for c in range(DC):
                        nc.tensor.matmul(
                            pv,
                            xT_sb[:, c, j * P : (j + 1) * P],
                            wv_sb[:, c, :],
                            start=(c == 0),
                            stop=(c == DC - 1),
                        )
                    nc.vector.tensor_copy(v_sb[:, j, 0:H], pv)

                    if j > 0:
                        emit_c_group(j - 1)
                emit_c_group(TB - 1)

    _strip_self_waits(nc)
    nc.finalize()  # Bacc.compile(): wait legalization + register allocation
    return nc


def _strip_self_waits(nc):
    """Drop same-engine semaphore waits on in-order engines (PE/ACT/DVE
    execute and complete strictly in order, so a self-wait is redundant).
    Tile emits them conservatively; walrus allows only one sem wait per
    compute instruction, and these push some matmuls/tensor-ops over."""
    prefixes = {"PE": "PE_", "Activation": "Activation_", "DVE": "DVE_"}
    for bb in nc.m.functions[0].blocks:
        for inst in bb.instructions:
            si = inst.sync_info
            if not si or not si.on_wait:
                continue
            pref = prefixes.get(str(inst.engine).split(".")[-1])
            if pref is None:
                continue
            keep = [w for w in si.on_wait if not (w.ant_name or "").startswith(pref)]
            if len(keep) != len(si.on_wait):
                si.on_wait = keep
                inst.sync_info = si


def kernel(**inputs):
    global LAST_RESULT
    x = np.asarray(inputs["x"], dtype=np.float32)
    bf = ml_dtypes.bfloat16
    w_bf = {
        k: np.asarray(inputs[k], dtype=np.float32).astype(bf)
        for k in ("Wq", "Wk", "Wv")
    }
    # dw[p=k_local, f=q_local] keeps entries with k <= q
    mask01 = (
        (np.arange(P)[:, None] <= np.arange(P)[None, :]).astype(np.float32).astype(bf)
    )

    if "nc" not in _CACHE:
        _CACHE["nc"] = _build()
    nc = _CACHE["nc"]

    from concourse.bass_utils import run_bass_kernel_spmd

    in_maps = [
        {
            "xT": np.ascontiguousarray(x[b].T).astype(bf),
            "Wq": w_bf["Wq"],
            "Wk": w_bf["Wk"],
            "Wv": w_bf["Wv"],
            "mask": mask01,
        }
        for b in range(B)
    ]
    res = run_bass_kernel_spmd(nc, in_maps, core_ids=list(range(B)))
    LAST_RESULT = res
    return np.stack([res.results[b]["out"] for b in range(B)]).astype(np.float32)
